# revision 12
# baseline (speedup 1.0000x reference)
"""ConcatRelationModule Bass kernel for 8 trn2 NeuronCores.

Reference computation (per edge e in [0, 16383)):
    x      = concat(inputs[heads[e], 0, :], inputs[e + 1, 1, :])     # [512]
    h      = tanh(concat(x @ W_FOH, x @ W_FOM) + b1)                 # [1024]
    h2     = tanh(h @ W2 + b2)                                       # [256]
    out[e] = h2 @ W3 + b3                                            # [64]

Strategy: data-parallel over edges (2048 per core, last edge padded).
Features live on SBUF partitions, edges on the free dim.  The modifier
half of x is a contiguous slice, so the host supplies it pre-transposed
(bwdT) and it DMAs straight into feature-major layout; only the gathered
head half needs PE transposes (2 per 128-edge subtile, down from 4).
L1 accumulates its k-chunks in order [2,3,0,1] so the dense bwd chunks
never wait on the gather.  The first 256-edge group runs its dense L1
half immediately after the preamble (partial sums spilled to SBUF by the
DVE and re-added in PSUM later), hiding the heads-table + first-gather
latency; a dummy tanh preloads the ACT table during the DMA ramp.
Weight loads ride the otherwise-idle Scalar engine queue (the dense w1
chunks split in halves so the first matmul starts after 128KB lands),
the lead group's modifier tiles ride the empty GpSimd queue ahead of the
gathers, mid-kernel output DMAs ride Sync, and the last group's output
DMA issues straight from Scalar after its ACT.  Short dependency-free
transpose chains warm the PE p-state during the DMA ramp and bridge the
fixed ~1.6us window while the second modifier tile is still in its DMA
queue.  Output is produced as bf16 [64, E] per core and converted to
f32 [E, 64] on host.
"""

import os

import numpy as np
import ml_dtypes

import concourse.bass as bass
import concourse.bacc as bacc
import concourse.mybir as mybir
import concourse.tile as tile
from concourse.bass import IndirectOffsetOnAxis
from concourse.bass_utils import run_bass_kernel_spmd
from concourse.masks import make_identity

N_TOKENS = 16384
LD = 256          # ldims
HID = 512
HID2 = 256
NREL = 64
NCORES = 8
E = N_TOKENS - 1  # 16383 real edges
EPC = N_TOKENS // NCORES  # 2048 edges per core (padded)
P = 128
SUBTILES = EPC // P       # 16 subtiles of 128 edges

# matmul operand dtype ("bf16" or "f32")
RUN_DT = os.environ.get("KERNEL_DT", "bf16")

LAST_RESULTS = None
_CACHE = {}

# small leading groups (dense-first) so the PE starts right after the
# preamble; small trailing groups to shorten the final ACT + out-DMA
# chain before the end-of-kernel barrier
GROUPS = [(0, 256), (256, 256), (512, 256), (768, 256), (1024, 512),
          (1536, 256), (1792, 128), (1920, 128)]
LEAD = (0,)               # group whose dense L1 half runs up front
KORDER = (2, 3, 0, 1)     # L1 k-chunk order: dense bwd chunks first


def _build(dt_str):
    cdt = mybir.dt.bfloat16 if dt_str == "bf16" else mybir.dt.float32
    f32 = mybir.dt.float32

    nc = bacc.Bacc()
    fwd = nc.declare_dram_parameter("fwd", [N_TOKENS, LD], cdt, isOutput=False)
    bwdT = nc.declare_dram_parameter("bwdT", [LD, EPC], cdt, isOutput=False)
    headsT = nc.declare_dram_parameter(
        "headsT", [P, SUBTILES], mybir.dt.int32, isOutput=False)
    w1 = nc.declare_dram_parameter("w1", [2 * LD, 2 * HID], cdt, isOutput=False)
    w2 = nc.declare_dram_parameter("w2", [2 * HID, HID2], cdt, isOutput=False)
    w3 = nc.declare_dram_parameter("w3", [HID2, NREL], cdt, isOutput=False)
    b1 = nc.declare_dram_parameter("b1", [P, 8], f32, isOutput=False)
    b2 = nc.declare_dram_parameter("b2", [P, 2], f32, isOutput=False)
    b3 = nc.declare_dram_parameter("b3", [NREL, 1], f32, isOutput=False)
    outT = nc.declare_dram_parameter("outT", [NREL, EPC], cdt, isOutput=True)

    Tanh = mybir.ActivationFunctionType.Tanh
    Identity = mybir.ActivationFunctionType.Identity

    with tile.TileContext(nc) as tc:
        with (
            tc.tile_pool(name="const", bufs=1) as const_pool,
            tc.tile_pool(name="xh", bufs=6) as xh_pool,
            tc.tile_pool(name="xTg", bufs=6) as xTg_pool,
            tc.tile_pool(name="xTd", bufs=8) as xTd_pool,
            tc.tile_pool(name="dsb", bufs=2) as dsb_pool,
            tc.tile_pool(name="h1", bufs=16) as h1_pool,
            tc.tile_pool(name="h2", bufs=4) as h2_pool,
            tc.tile_pool(name="outs", bufs=2) as out_pool,
            tc.tile_pool(name="pt", bufs=2, space="PSUM") as pt_pool,
            tc.tile_pool(name="ph", bufs=4, space="PSUM") as ph_pool,
            tc.tile_pool(name="pj", bufs=2, space="PSUM") as pj_pool,
        ):
            # heads table first: the whole gather stream waits on it
            hT_sb = const_pool.tile([P, SUBTILES], mybir.dt.int32)
            nc.sync.dma_start(hT_sb[:], headsT[:])
            ident = const_pool.tile([P, P], cdt)
            make_identity(nc, ident[:])
            # p-state warmup: dependency-free transposes keep the PE busy
            # from ~7.8us so the 0.65->2.4GHz clock ramp (needs ~3us of
            # continuous execution) completes before the first real matmul
            wpt = pt_pool.tile([P, P], cdt, tag="pt", name="warm_pt")
            for _ in range(16):
                nc.tensor.transpose(wpt[:], ident[:], ident[:])

            xh_tiles = [None] * len(GROUPS)
            xd_tiles = [None] * len(GROUPS)

            def load_group(gi):
                start, size = GROUPS[gi]
                ns = size // P
                # gathered head rows, edge-major (transposed on PE later)
                xh = xh_pool.tile([P, ns, LD], cdt, tag="xh", name=f"xh_{gi}")
                for s in range(ns):
                    t = start // P + s
                    nc.gpsimd.indirect_dma_start(
                        out=xh[:, s, :],
                        out_offset=None,
                        in_=fwd[:],
                        in_offset=IndirectOffsetOnAxis(ap=hT_sb[:, t:t + 1], axis=0),
                    )
                # modifier rows arrive pre-transposed from the host: two
                # feature-major tiles, no PE transpose needed
                xd2 = xTd_pool.tile([P, size], cdt, tag="xTd", name=f"xd2_{gi}")
                xd3 = xTd_pool.tile([P, size], cdt, tag="xTd", name=f"xd3_{gi}")
                deng = nc.gpsimd if gi in LEAD else nc.sync
                deng.dma_start(xd2[:], bwdT[0:P, start:start + size])
                deng.dma_start(xd3[:], bwdT[P:2 * P, start:start + size])
                xh_tiles[gi] = xh
                xd_tiles[gi] = (xd2, xd3)

            load_group(0)
            load_group(1)
            # w1 per k-chunk on the scalar queue (sync is busy with headsT +
            # lead xd loads); dense chunks (2,3) first since L1 uses them
            # first
            w1_sb = [None] * 4
            for kc in (2, 3, 0, 1):
                w1_sb[kc] = const_pool.tile(
                    [P, 2 * HID], cdt, tag=f"w1_{kc}", name=f"w1_{kc}")
            for kc, half in ((2, 0), (3, 0), (2, 1), (3, 1)):
                nc.scalar.dma_start(
                    w1_sb[kc][:, half * HID:(half + 1) * HID],
                    w1[kc * P:(kc + 1) * P, half * HID:(half + 1) * HID])
            for kc in (0, 1):
                nc.sync.dma_start(w1_sb[kc][:], w1[kc * P:(kc + 1) * P, :])
            b1_sb = const_pool.tile([P, 8], f32)
            nc.scalar.dma_start(b1_sb[:], b1[:])
            # dummy tanh: pull the ~1.3us ACT table load into the DMA ramp,
            # off the critical path of the first real activation
            warm = const_pool.tile([P, 1], f32)
            nc.scalar.activation(out=warm[:], in_=b1_sb[:, 0:1], func=Tanh)
            load_group(2)
            load_group(3)
            w2_sb = const_pool.tile([P, 8, HID2], cdt)
            nc.scalar.dma_start(w2_sb[:], w2.rearrange("(kc p) j -> p kc j", p=P))
            b2_sb = const_pool.tile([P, 2], f32)
            nc.scalar.dma_start(b2_sb[:], b2[:])
            load_group(4)
            w3_sb = const_pool.tile([P, 2, NREL], cdt)
            nc.scalar.dma_start(w3_sb[:], w3.rearrange("(kc p) r -> p kc r", p=P))
            b3_sb = const_pool.tile([NREL, 1], f32)
            nc.scalar.dma_start(b3_sb[:], b3[:])
            load_group(5)
            load_group(6)
            load_group(7)

            # ---- dense-first prelude for the lead groups: run the bwd
            # half of L1 while the heads table + first gathers are still
            # in flight; spill the partial sums to SBUF via the DVE ----
            dsb_tiles = {}
            for gi in LEAD:
                start, size = GROUPS[gi]
                xd2, xd3 = xd_tiles[gi]
                dsb = dsb_pool.tile([P, 8, size], f32, tag="dsb",
                                    name=f"dsb_{gi}")
                # two half-passes of 4 hc: all kc2 matmuls fire as soon as
                # xd2 lands, and a dependency-free transpose bridge fills
                # the PE while xd3 (second in its DMA queue, ~1.6us later)
                # is still in flight
                for half in range(2):
                    hcs = range(half * 4, half * 4 + 4)
                    phs = {}
                    for hc in hcs:
                        phs[hc] = ph_pool.tile([P, size], f32, tag="ph",
                                               name=f"phd_{gi}_{hc}")
                        nc.tensor.matmul(
                            out=phs[hc][:],
                            lhsT=w1_sb[2][:, hc * P:(hc + 1) * P],
                            rhs=xd2[:], start=True, stop=False)
                    if half == 0:
                        wpt2 = pt_pool.tile([P, P], cdt, tag="pt",
                                            name="bridge_pt")
                        for _ in range(12):
                            nc.tensor.transpose(wpt2[:], ident[:], ident[:])
                    for hc in hcs:
                        nc.tensor.matmul(
                            out=phs[hc][:],
                            lhsT=w1_sb[3][:, hc * P:(hc + 1) * P],
                            rhs=xd3[:], start=False, stop=True)
                    for hc in hcs:
                        nc.vector.tensor_copy(out=dsb[:, hc, :],
                                              in_=phs[hc][:])
                dsb_tiles[gi] = dsb

            # second bridge: the first gather lands ~1.3us after the
            # prelude drains; keep the PE (and its p-state) busy till then
            wpt3 = pt_pool.tile([P, P], cdt, tag="pt", name="gap_pt")
            for _ in range(10):
                nc.tensor.transpose(wpt3[:], ident[:], ident[:])

            xTg_tiles = [None] * len(GROUPS)

            def emit_transpose(gi):
                start, size = GROUPS[gi]
                ns = size // P
                xh = xh_tiles[gi]
                xTs = []
                for kc in range(2):
                    col = kc * P
                    pt = pt_pool.tile([P, size], cdt, tag="pt",
                                      name=f"pt_{gi}_{kc}")
                    for s in range(ns):
                        nc.tensor.transpose(
                            pt[:, s * P:(s + 1) * P],
                            xh[:, s, col:col + P], ident[:])
                    xT = xTg_pool.tile([P, size], cdt, tag="xTg",
                                       name=f"xT_{gi}_{kc}")
                    nc.vector.tensor_copy(out=xT[:], in_=pt[:])
                    xTs.append(xT)
                xTg_tiles[gi] = xTs

            emit_transpose(0)
            for gi, (start, size) in enumerate(GROUPS):
                xg = xTg_tiles[gi]
                xd2, xd3 = xd_tiles[gi]
                xTs = {0: xg[0], 1: xg[1], 2: xd2, 3: xd3}
                # ---- layer 1: h = tanh(W1.T-chunks @ x + b1), 8 h-chunks ----
                h1s = []
                for hc in range(8):
                    ph = ph_pool.tile([P, size], f32, tag="ph",
                                      name=f"ph_{gi}_{hc}")
                    if gi in LEAD:
                        # dense half already spilled to SBUF; only the
                        # gathered chunks run here, then the DVE re-adds
                        for i, kc in enumerate((0, 1)):
                            nc.tensor.matmul(
                                out=ph[:],
                                lhsT=w1_sb[kc][:, hc * P:(hc + 1) * P],
                                rhs=xTs[kc][:],
                                start=(i == 0), stop=(i == 1))
                        nc.vector.tensor_add(
                            out=ph[:], in0=ph[:], in1=dsb_tiles[gi][:, hc, :])
                    else:
                        for i, kc in enumerate(KORDER):
                            nc.tensor.matmul(
                                out=ph[:],
                                lhsT=w1_sb[kc][:, hc * P:(hc + 1) * P],
                                rhs=xTs[kc][:],
                                start=(i == 0), stop=(i == 3))
                    h1 = h1_pool.tile([P, size], cdt, tag="h1",
                                      name=f"h1_{gi}_{hc}")
                    nc.scalar.activation(
                        out=h1[:], in_=ph[:], func=Tanh,
                        bias=b1_sb[:, hc:hc + 1],
                    )
                    h1s.append(h1)

                # transpose the NEXT group now so its DVE copies finish
                # while this group's L2/L3 run on the PE
                if gi + 1 < len(GROUPS):
                    emit_transpose(gi + 1)

                # ---- layer 2: h2 = tanh(W2-chunks @ h + b2), 2 j-chunks ----
                h2s = []
                for jc in range(2):
                    pj = pj_pool.tile([P, size], f32, tag="pj",
                                      name=f"pj_{gi}_{jc}")
                    for kc in range(8):
                        nc.tensor.matmul(
                            out=pj[:],
                            lhsT=w2_sb[:, kc, jc * P:(jc + 1) * P],
                            rhs=h1s[kc][:],
                            start=(kc == 0),
                            stop=(kc == 7),
                        )
                    h2 = h2_pool.tile([P, size], cdt, tag="h2",
                                      name=f"h2_{gi}_{jc}")
                    nc.scalar.activation(
                        out=h2[:], in_=pj[:], func=Tanh,
                        bias=b2_sb[:, jc:jc + 1],
                    )
                    h2s.append(h2)

                # ---- layer 3: out = W3-chunks @ h2 + b3 ----
                po = pj_pool.tile([NREL, size], f32, tag="pj", name=f"po_{gi}")
                for kc in range(2):
                    nc.tensor.matmul(
                        out=po[:],
                        lhsT=w3_sb[:, kc, :],
                        rhs=h2s[kc][:],
                        start=(kc == 0),
                        stop=(kc == 1),
                    )
                o = out_pool.tile([NREL, size], cdt, tag="o", name=f"o_{gi}")
                nc.scalar.activation(
                    out=o[:], in_=po[:], func=Identity, bias=b3_sb[:, 0:1]
                )
                # mid-kernel out DMAs ride sync (Scalar stays clear for
                # ACTs); the last group's rides scalar: no cross-engine
                # handoff on the tail chain
                oeng = nc.scalar if gi == len(GROUPS) - 1 else nc.sync
                oeng.dma_start(outT[:, start:start + size], o[:])

    nc.finalize()
    return nc


def kernel(inputs, rhidLayerFOH, rhidLayerFOM, rcatBias, rhid2Layer, rhid2Bias,
           routLayer, routBias, heads):
    global LAST_RESULTS

    inputs = np.asarray(inputs, dtype=np.float32)
    heads = np.asarray(heads)

    if RUN_DT == "bf16":
        wdt = ml_dtypes.bfloat16
    else:
        wdt = np.float32

    fwd = np.ascontiguousarray(inputs[:, 0, :]).astype(wdt)      # [N, 256]
    bwd_full = inputs[:, 1, :]                                   # [N, 256]
    # mods for edge e is e+1; pad edge 16383 with mod 16383 (garbage, dropped)
    mods_pad = np.concatenate([np.arange(1, N_TOKENS), [N_TOKENS - 1]]).astype(np.int64)
    heads_pad = np.concatenate([heads.astype(np.int64), [0]]).astype(np.int32)

    w1 = np.ascontiguousarray(
        np.concatenate([np.asarray(rhidLayerFOH), np.asarray(rhidLayerFOM)], axis=1)
    ).astype(wdt)                                                # [512, 1024]
    w2 = np.ascontiguousarray(np.asarray(rhid2Layer)).astype(wdt)  # [1024, 256]
    w3 = np.ascontiguousarray(np.asarray(routLayer)).astype(wdt)   # [256, 64]
    b1 = np.ascontiguousarray(
        np.asarray(rcatBias, dtype=np.float32).reshape(8, P).T)    # [128, 8]
    b2 = np.ascontiguousarray(
        np.asarray(rhid2Bias, dtype=np.float32).reshape(2, P).T)   # [128, 2]
    b3 = np.ascontiguousarray(
        np.asarray(routBias, dtype=np.float32).reshape(1, NREL).T)  # [64, 1]

    in_maps = []
    for c in range(NCORES):
        sl = slice(c * EPC, (c + 1) * EPC)
        # modifier rows for this core, pre-transposed to feature-major
        bwdT_c = np.ascontiguousarray(
            bwd_full[mods_pad[sl]].T).astype(wdt)                 # [256, 2048]
        headsT_c = np.ascontiguousarray(
            heads_pad[sl].reshape(SUBTILES, P).T)                 # [128, 16]
        in_maps.append({
            "fwd": fwd, "bwdT": bwdT_c, "headsT": headsT_c,
            "w1": w1, "w2": w2, "w3": w3, "b1": b1, "b2": b2, "b3": b3,
        })

    if RUN_DT not in _CACHE:
        _CACHE[RUN_DT] = _build(RUN_DT)
    nc = _CACHE[RUN_DT]

    trace_dir = os.environ.get("KERNEL_TRACE_DIR") or None
    res = run_bass_kernel_spmd(nc, in_maps, list(range(NCORES)), tmpdir=trace_dir)
    LAST_RESULTS = res

    outT = np.concatenate([r["outT"] for r in res.results], axis=1)  # [64, 16384]
    return np.ascontiguousarray(outT.T[:E]).astype(np.float32)       # [16383, 64]
